# revision 47
# baseline (speedup 1.0000x reference)
"""GSphereNet message-passing layer on 8 TRN2 NeuronCores (Bass/Tile).

Math: out = x + relu((segsum(feat97) @ W_aug) @ W1 + b1) @ W2 + b2
where feat97 = [rbf|angle|1] per edge -- the edge projection commutes with
segment_sum, so aggregation moves 97 floats/edge, and W_aug @ W1 folds into
a single [97,512] matrix on host (the device MLP is two GEMMs).

Distribution: edges are routed BY DESTINATION SHARD on host -- core c gets
exactly the edges targeting its 6272-node slice, so there is no collective
at all. Per 128-node chunk, segment-sum runs as a one-hot matmul on the
TensorEngine: aggT[feat, node] = sum_tiles F_tile[128e,104f].T @ S[128e,128n]
with S = (iota == dest) built by one batched DVE compare per chunk (emitted
two node-groups ahead so the DVE never gates the PE). The residual x+b2 is
injected into the out-GEMM PSUM via an identity matmul. bf16 operands and
output, fp32 PSUM accumulation (measured rel-err 4.6e-3 vs the 2e-2 gate).
Measured HW exec ~126 us on 8 cores vs 1.585 ms for the scatter-add +
ReduceScatter baseline (~12.6x).
"""
import sys

sys.path.insert(0, '/opt/trn_rl_repo')

import os
import numpy as np
from ml_dtypes import bfloat16

P = 128
N_NODES = 50000
N_EDGES = 400000
EMBED = 512
RBF = 64
ANG = 32
N_CORES = 8

NODES_PAD = 50176              # 8 * 6272
RPC = NODES_PAD // N_CORES     # 6272 rows per core
NCH = RPC // P                 # 49 chunks of 128 nodes per core
FEAT = 104                     # 97 used features padded to 8-elem alignment
NCHUNK = 512                   # node-MLP group (4 chunks)
GT = 64                        # token tiles per DMA group
SUB = 128                      # one-hot window: nodes per aggregation subchunk
NSUB = RPC // SUB              # 98 subchunks per core


def _host_pack(rows, rbf_feature, angle_feature):
    """Route edges by destination shard; per 128-node chunk pad to whole
    128-edge tiles with a tile schedule shared by all cores (SPMD).
    Returns (tok_list, dst_list, ntiles)."""
    order = np.argsort(rows, kind='stable')
    rs = rows[order]
    gstart = np.searchsorted(rs, np.arange(0, NODES_PAD, SUB))  # subchunk starts
    gcnt = np.diff(np.r_[gstart, N_EDGES])
    counts = gcnt.reshape(N_CORES, NSUB)                      # [core, subchunk]
    tiles_per_chunk = np.maximum(1, -(-counts.max(axis=0) // P))  # [NSUB]
    offs = np.concatenate([[0], np.cumsum(tiles_per_chunk)])
    ntiles = int(offs[-1])

    # position of each (sorted) edge inside its core's padded stream
    rank = np.arange(N_EDGES) - np.repeat(gstart, gcnt)
    gid = rs // SUB                    # global subchunk id
    sub = gid % NSUB
    core_s = gid // NSUB
    pos = offs[sub] * P + rank         # stream slot within core

    feat = np.zeros((N_EDGES, FEAT), dtype=np.float32)
    feat[:, :RBF] = rbf_feature
    feat[:, RBF:RBF + ANG] = angle_feature
    feat[:, 96] = 1.0
    featb = feat.astype(bfloat16)

    dloc = (rs % SUB).astype(np.float32)

    tok_list, dst_list = [], []
    for c in range(N_CORES):
        m = core_s == c
        stream = np.zeros((ntiles * P, FEAT), dtype=bfloat16)
        dstr = np.zeros((ntiles * P,), dtype=np.float32)
        stream[pos[m]] = featb[order[m]]
        dstr[pos[m]] = dloc[m]
        tok_list.append(np.ascontiguousarray(
            stream.reshape(ntiles, P, FEAT).transpose(1, 0, 2)))
        dst_list.append(np.ascontiguousarray(dstr.reshape(ntiles, P).T))
    return tok_list, dst_list, ntiles, tiles_per_chunk


def _build_program(ntiles, tiles_per_chunk, mybir, bacc, tile):
    f32 = mybir.dt.float32
    f16 = mybir.dt.float16
    bf16 = mybir.dt.bfloat16
    nc = bacc.Bacc("TRN2", target_bir_lowering=False, debug=False,
                   num_devices=N_CORES)
    tok_d = nc.dram_tensor("tokens", [P, ntiles, FEAT], bf16, kind="ExternalInput")
    dst_d = nc.dram_tensor("dests", [P, ntiles], f32, kind="ExternalInput")
    xb_d = nc.dram_tensor("xb", [P, NCH, EMBED], bf16, kind="ExternalInput")
    wc_d = nc.dram_tensor("wc", [FEAT, EMBED], bf16, kind="ExternalInput")
    w2_d = nc.dram_tensor("w2d", [P, 4, EMBED], bf16, kind="ExternalInput")
    b1_d = nc.dram_tensor("b1t", [P, 4], f32, kind="ExternalInput")
    out_d = nc.dram_tensor("out_s", [P, NCH, EMBED], bf16, kind="ExternalOutput")
    iota_d = nc.inline_tensor(
        np.tile(np.arange(SUB, dtype=np.float32), (P, 1)), "iota")
    ident_d = nc.inline_tensor(np.eye(P, dtype=np.float32).astype(bfloat16),
                               "ident")

    offs = np.concatenate([[0], np.cumsum(tiles_per_chunk)])
    act_relu = mybir.ActivationFunctionType.Relu
    is_eq = mybir.AluOpType.is_equal
    add_op = mybir.AluOpType.add

    with tile.TileContext(nc) as tc:
        with (
            tc.tile_pool(name="wts", bufs=1) as wpool,
            tc.tile_pool(name="toks", bufs=6) as fpool,
            tc.tile_pool(name="sones", bufs=14) as spool,
            tc.tile_pool(name="aggt", bufs=1) as apool,
            tc.tile_pool(name="hts", bufs=3) as hpool,
            tc.tile_pool(name="xin", bufs=2) as xpool,
            tc.tile_pool(name="oout", bufs=3) as opool,
            tc.tile_pool(name="psa", bufs=2, space="PSUM") as psa,
            tc.tile_pool(name="psh", bufs=2, space="PSUM") as psh,
            tc.tile_pool(name="pso", bufs=3, space="PSUM") as pso,
        ):
            # token-group boundaries: small first group so the PE can start
            # quickly, then full-size groups
            g_bounds = [0, min(16, ntiles)]
            while g_bounds[-1] < ntiles:
                g_bounds.append(min(g_bounds[-1] + GT, ntiles))
            loaded = {}

            def load_group(g):
                ft = fpool.tile([P, GT, FEAT], bf16, tag="fg")
                gsz = g_bounds[g + 1] - g_bounds[g]
                nc.sync.dma_start(
                    out=ft[:, :gsz, :],
                    in_=tok_d[:, g_bounds[g]:g_bounds[g + 1], :])
                loaded[g] = ft

            def tok_ap(t):
                g = next(i for i in range(len(g_bounds) - 1)
                         if g_bounds[i] <= t < g_bounds[i + 1])
                if g not in loaded:
                    load_group(g)
                return loaded[g][:, t - g_bounds[g], :]

            # PE warm-up: dummy matmuls on a zeroed tile keep the HAM clock
            # at 8/8 while the first token DMAs land
            warm = wpool.tile([P, P], bf16)
            nc.vector.memset(warm[:], 0.0)
            pwarm = psh.tile([P, EMBED], f32, tag="h")
            for _ in range(20):
                nc.tensor.matmul(pwarm[:, :P], warm[:], warm[:],
                                 start=True, stop=True)
            # agg-critical loads first, then MLP weights
            load_group(0)
            dst = wpool.tile([P, ntiles], f32)
            nc.sync.dma_start(out=dst[:], in_=dst_d[:])
            iota = wpool.tile([P, SUB], f32)
            nc.sync.dma_start(out=iota[:], in_=iota_d[:])
            for g in range(1, min(5, len(g_bounds) - 1)):
                load_group(g)
            wc = wpool.tile([FEAT, EMBED], bf16)
            nc.sync.dma_start(out=wc[:], in_=wc_d[:])
            w2 = wpool.tile([P, 4, EMBED], bf16)
            nc.sync.dma_start(out=w2[:], in_=w2_d[:])
            b1t = wpool.tile([P, 4], f32)
            nc.sync.dma_start(out=b1t[:], in_=b1_d[:])
            ident = wpool.tile([P, P], bf16)
            nc.sync.dma_start(out=ident[:], in_=ident_d[:])
            aggT = apool.tile([FEAT, RPC], bf16)
            maxt = int(tiles_per_chunk.max())

            n_groups = -(-RPC // NCHUNK)
            s3map = {}

            def emit_s3_group(gj):
                # build the one-hots for group gj's subchunks ahead of time
                # so the DVE never sits between PE's out-GEMMs and the next
                # group's agg matmuls
                nj0 = gj * NCHUNK
                njw = min(NCHUNK, RPC - nj0)
                for sci in range(nj0 // SUB, (nj0 + njw) // SUB):
                    t0, t1 = int(offs[sci]), int(offs[sci + 1])
                    nt = t1 - t0
                    S3 = spool.tile([P, maxt, SUB], bf16, tag="S")
                    nc.vector.tensor_tensor(
                        S3[:, :nt, :],
                        iota[:].unsqueeze(1).broadcast_to([P, nt, SUB]),
                        dst[:, t0:t1].unsqueeze(2).broadcast_to([P, nt, SUB]),
                        is_eq)
                    s3map[sci] = S3

            emit_s3_group(0)
            emit_s3_group(1)
            for gi in range(n_groups):
                if gi + 2 < n_groups:
                    emit_s3_group(gi + 2)
                n0 = gi * NCHUNK
                nw = min(NCHUNK, RPC - n0)
                # ---- aggregate this node group, one 64-node window at a
                # time (edges are sorted, so each token tile hits only one
                # window; the one-hot is 64 wide and the matmul streams
                # just 64 columns) ----
                ps4 = psa.tile([FEAT, NCHUNK // P, P], f32, tag="agg")
                for sci in range(n0 // SUB, (n0 + nw) // SUB):
                    si = (sci * SUB - n0) // P
                    so = (sci * SUB) % P
                    t0, t1 = int(offs[sci]), int(offs[sci + 1])
                    S3 = s3map.pop(sci)
                    for t in range(t0, t1):
                        fa = tok_ap(t)
                        nc.tensor.matmul(ps4[:, si, so:so + SUB], fa,
                                         S3[:, t - t0, :],
                                         start=(t == t0), stop=(t == t1 - 1))
                nc.scalar.copy(out=aggT[:, n0:n0 + nw],
                               in_=ps4[:, :nw // P, :])
                # ---- node MLP on this group ----
                hT = hpool.tile([P, 4, NCHUNK], bf16, tag="hT")
                for f in range(4):
                    ph = psh.tile([P, NCHUNK], f32, tag="h")
                    nc.tensor.matmul(ph[:, :nw], wc[:, f * P:(f + 1) * P],
                                     aggT[:, n0:n0 + nw], start=True, stop=True)
                    nc.scalar.activation(out=hT[:, f, :nw], in_=ph[:, :nw],
                                         func=act_relu, bias=b1t[:, f:f + 1])
                c0 = n0 // P
                xt = xpool.tile([P, NCHUNK // P, EMBED], bf16, tag="x")
                nc.sync.dma_start(
                    out=xt[:, :nw // P, :],
                    in_=xb_d[:, c0:c0 + nw // P, :])
                ot = opool.tile([P, NCHUNK // P, EMBED], bf16, tag="o")
                for t in range(nw // P):
                    po = pso.tile([P, EMBED], f32, tag="po")
                    nc.tensor.matmul(po[:], ident[:], xt[:, t, :],
                                     start=True, stop=False)
                    for k in range(4):
                        nc.tensor.matmul(po[:], hT[:, k, t * P:(t + 1) * P],
                                         w2[:, k, :],
                                         start=False, stop=(k == 3))
                    nc.scalar.copy(out=ot[:, t, :], in_=po[:])
                nc.gpsimd.dma_start(
                    out=out_d[:, c0:c0 + nw // P, :],
                    in_=ot[:, :nw // P, :])
    nc.finalize()
    return nc


def kernel(x, edge_index, rbf_feature, angle_feature, W_edge, b_edge, W1, b1, W2, b2):
    from concourse import bacc, tile, mybir
    from concourse.bass_utils import run_bass_kernel_spmd

    x = np.asarray(x, dtype=np.float32)
    rows = np.asarray(edge_index[0], dtype=np.int64)
    tok_list, dst_list, ntiles, tiles_per_chunk = _host_pack(
        rows, np.asarray(rbf_feature, np.float32),
        np.asarray(angle_feature, np.float32))

    # fold W_aug @ W1 on host (fp64), plus x+b2
    W_aug = np.zeros((FEAT, EMBED), dtype=np.float64)
    W_aug[:RBF + ANG] = np.asarray(W_edge, np.float64)
    W_aug[96] = np.asarray(b_edge, np.float64)
    wc = (W_aug @ np.asarray(W1, np.float64)).astype(np.float32).astype(bfloat16)
    w2d = np.ascontiguousarray(
        np.asarray(W2, np.float32).reshape(4, P, EMBED).transpose(1, 0, 2)
    ).astype(bfloat16)
    b1t = np.ascontiguousarray(np.asarray(b1, np.float32).reshape(4, P).T)
    xb_full = np.zeros((NODES_PAD, EMBED), dtype=bfloat16)
    xb_full[:N_NODES] = (x + np.asarray(b2, np.float32)).astype(bfloat16)

    in_maps = []
    for c in range(N_CORES):
        xbs = xb_full[c * RPC:(c + 1) * RPC]
        in_maps.append({
            "tokens": tok_list[c], "dests": dst_list[c],
            "xb": np.ascontiguousarray(
                xbs.reshape(NCH, P, EMBED).transpose(1, 0, 2)),
            "wc": wc, "w2d": w2d, "b1t": b1t,
        })

    nc = _build_program(ntiles, tiles_per_chunk, mybir, bacc, tile)
    if os.environ.get('DEBUG_SIM') == '1':
        from concourse import bass_interp
        results = []
        for c in range(N_CORES):
            sim = bass_interp.CoreSim(nc)
            for k, v in in_maps[c].items():
                sim.tensor(k)[:] = v
            sim.tensor('out_s')[:] = 0
            sim.simulate()
            results.append({'out_s': np.array(sim.tensor('out_s'))})

        class R:
            pass
        res = R()
        res.results = results
        res.exec_time_ns = None
        res.instructions_and_trace = None
        res.profile_json = None
    else:
        res = run_bass_kernel_spmd(nc, in_maps, list(range(N_CORES)))
    global LAST_RESULT
    LAST_RESULT = res
    out = np.concatenate(
        [res.results[c]["out_s"].astype(np.float32)
         .transpose(1, 0, 2).reshape(RPC, EMBED)
         for c in range(N_CORES)], axis=0)
    return out[:N_NODES]


LAST_RESULT = None


# revision 49
# speedup vs baseline: 1.0624x; 1.0624x over previous
"""GSphereNet message-passing layer on 8 TRN2 NeuronCores (Bass/Tile).

Math: out = x + relu((segsum(feat97) @ W_aug) @ W1 + b1) @ W2 + b2
where feat97 = [rbf|angle|1] per edge -- the edge projection commutes with
segment_sum, so aggregation moves 97 floats/edge, and W_aug @ W1 folds into
a single [97,512] matrix on host (the device MLP is two GEMMs).

Distribution: edges are routed BY DESTINATION SHARD on host -- core c gets
exactly the edges targeting its 6272-node slice, so there is no collective
at all. Per 128-node chunk, segment-sum runs as a one-hot matmul on the
TensorEngine: aggT[feat, node] = sum_tiles F_tile[128e,104f].T @ S[128e,128n]
with S = (iota == dest) built by one batched DVE compare per chunk (emitted
two node-groups ahead so the DVE never gates the PE). The residual x+b2 is
injected into the out-GEMM PSUM via an identity matmul. bf16 operands and
output, fp32 PSUM accumulation (measured rel-err 4.6e-3 vs the 2e-2 gate).
Measured HW exec ~126 us on 8 cores vs 1.585 ms for the scatter-add +
ReduceScatter baseline (~12.6x).
"""
import sys

sys.path.insert(0, '/opt/trn_rl_repo')

import os
import numpy as np
from ml_dtypes import bfloat16

P = 128
N_NODES = 50000
N_EDGES = 400000
EMBED = 512
RBF = 64
ANG = 32
N_CORES = 8

NODES_PAD = 50176              # 8 * 6272
RPC = NODES_PAD // N_CORES     # 6272 rows per core
NCH = RPC // P                 # 49 chunks of 128 nodes per core
FEAT = 104                     # 97 used features padded to 8-elem alignment
NCHUNK = 512                   # node-MLP group (4 chunks)
GT = 64                        # token tiles per DMA group
SUB = 128                      # one-hot window: nodes per aggregation subchunk
NSUB = RPC // SUB              # 98 subchunks per core


def _host_pack(rows, rbf_feature, angle_feature):
    """Route edges by destination shard; per 128-node chunk pad to whole
    128-edge tiles with a tile schedule shared by all cores (SPMD).
    Returns (tok_list, dst_list, ntiles)."""
    order = np.argsort(rows, kind='stable')
    rs = rows[order]
    gstart = np.searchsorted(rs, np.arange(0, NODES_PAD, SUB))  # subchunk starts
    gcnt = np.diff(np.r_[gstart, N_EDGES])
    counts = gcnt.reshape(N_CORES, NSUB)                      # [core, subchunk]
    tiles_per_chunk = np.maximum(1, -(-counts.max(axis=0) // P))  # [NSUB]
    offs = np.concatenate([[0], np.cumsum(tiles_per_chunk)])
    ntiles = int(offs[-1])

    # position of each (sorted) edge inside its core's padded stream
    rank = np.arange(N_EDGES) - np.repeat(gstart, gcnt)
    gid = rs // SUB                    # global subchunk id
    sub = gid % NSUB
    core_s = gid // NSUB
    pos = offs[sub] * P + rank         # stream slot within core

    feat = np.zeros((N_EDGES, FEAT), dtype=np.float32)
    feat[:, :RBF] = rbf_feature
    feat[:, RBF:RBF + ANG] = angle_feature
    feat[:, 96] = 1.0
    featb = feat.astype(bfloat16)

    dloc = (rs % SUB).astype(np.float32)

    tok_list, dst_list = [], []
    for c in range(N_CORES):
        m = core_s == c
        stream = np.zeros((ntiles * P, FEAT), dtype=bfloat16)
        dstr = np.zeros((ntiles * P,), dtype=np.float32)
        stream[pos[m]] = featb[order[m]]
        dstr[pos[m]] = dloc[m]
        tok_list.append(np.ascontiguousarray(
            stream.reshape(ntiles, P, FEAT).transpose(1, 0, 2)))
        dst_list.append(np.ascontiguousarray(dstr.reshape(ntiles, P).T))
    return tok_list, dst_list, ntiles, tiles_per_chunk


def _build_program(ntiles, tiles_per_chunk, mybir, bacc, tile):
    f32 = mybir.dt.float32
    f16 = mybir.dt.float16
    bf16 = mybir.dt.bfloat16
    nc = bacc.Bacc("TRN2", target_bir_lowering=False, debug=False,
                   num_devices=N_CORES)
    tok_d = nc.dram_tensor("tokens", [P, ntiles, FEAT], bf16, kind="ExternalInput")
    dst_d = nc.dram_tensor("dests", [P, ntiles], f32, kind="ExternalInput")
    xb_d = nc.dram_tensor("xb", [P, NCH, EMBED], bf16, kind="ExternalInput")
    wc_d = nc.dram_tensor("wc", [FEAT, EMBED], bf16, kind="ExternalInput")
    w2_d = nc.dram_tensor("w2d", [P, 4, EMBED], bf16, kind="ExternalInput")
    b1_d = nc.dram_tensor("b1t", [P, 4], f32, kind="ExternalInput")
    out_d = nc.dram_tensor("out_s", [P, NCH, EMBED], bf16, kind="ExternalOutput")
    iota_d = nc.inline_tensor(
        np.tile(np.arange(SUB, dtype=np.float32), (P, 1)), "iota")
    ident_d = nc.inline_tensor(np.eye(P, dtype=np.float32).astype(bfloat16),
                               "ident")

    offs = np.concatenate([[0], np.cumsum(tiles_per_chunk)])
    act_relu = mybir.ActivationFunctionType.Relu
    is_eq = mybir.AluOpType.is_equal
    add_op = mybir.AluOpType.add

    with tile.TileContext(nc) as tc:
        with (
            tc.tile_pool(name="wts", bufs=1) as wpool,
            tc.tile_pool(name="toks", bufs=6) as fpool,
            tc.tile_pool(name="sones", bufs=15) as spool,
            tc.tile_pool(name="aggt", bufs=1) as apool,
            tc.tile_pool(name="hts", bufs=3) as hpool,
            tc.tile_pool(name="xin", bufs=4) as xpool,
            tc.tile_pool(name="oout", bufs=3) as opool,
            tc.tile_pool(name="psa", bufs=2, space="PSUM") as psa,
            tc.tile_pool(name="psh", bufs=2, space="PSUM") as psh,
            tc.tile_pool(name="pso", bufs=3, space="PSUM") as pso,
        ):
            # token-group boundaries: small first group so the PE can start
            # quickly, then full-size groups
            g_bounds = [0, min(16, ntiles)]
            while g_bounds[-1] < ntiles:
                g_bounds.append(min(g_bounds[-1] + GT, ntiles))
            loaded = {}

            def load_group(g):
                ft = fpool.tile([P, GT, FEAT], bf16, tag="fg")
                gsz = g_bounds[g + 1] - g_bounds[g]
                nc.sync.dma_start(
                    out=ft[:, :gsz, :],
                    in_=tok_d[:, g_bounds[g]:g_bounds[g + 1], :])
                loaded[g] = ft

            def tok_ap(t):
                g = next(i for i in range(len(g_bounds) - 1)
                         if g_bounds[i] <= t < g_bounds[i + 1])
                if g not in loaded:
                    load_group(g)
                return loaded[g][:, t - g_bounds[g], :]

            # PE warm-up: dummy matmuls on a zeroed tile keep the HAM clock
            # at 8/8 while the first token DMAs land
            warm = wpool.tile([P, P], bf16)
            nc.vector.memset(warm[:], 0.0)
            pwarm = psh.tile([P, EMBED], f32, tag="h")
            for _ in range(20):
                nc.tensor.matmul(pwarm[:, :P], warm[:], warm[:],
                                 start=True, stop=True)
            # agg-critical loads first, then MLP weights
            load_group(0)
            dst = wpool.tile([P, ntiles], f32)
            nc.sync.dma_start(out=dst[:], in_=dst_d[:])
            iota = wpool.tile([P, SUB], f32)
            nc.sync.dma_start(out=iota[:], in_=iota_d[:])
            for g in range(1, min(5, len(g_bounds) - 1)):
                load_group(g)
            wc = wpool.tile([FEAT, EMBED], bf16)
            nc.sync.dma_start(out=wc[:], in_=wc_d[:])
            w2 = wpool.tile([P, 4, EMBED], bf16)
            nc.sync.dma_start(out=w2[:], in_=w2_d[:])
            b1t = wpool.tile([P, 4], f32)
            nc.sync.dma_start(out=b1t[:], in_=b1_d[:])
            ident = wpool.tile([P, P], bf16)
            nc.sync.dma_start(out=ident[:], in_=ident_d[:])
            aggT = apool.tile([FEAT, RPC], bf16)
            maxt = int(tiles_per_chunk.max())

            n_groups = -(-RPC // NCHUNK)
            s3map = {}

            def emit_s3_group(gj):
                # build the one-hots for group gj's subchunks ahead of time
                # so the DVE never sits between PE's out-GEMMs and the next
                # group's agg matmuls
                nj0 = gj * NCHUNK
                njw = min(NCHUNK, RPC - nj0)
                for sci in range(nj0 // SUB, (nj0 + njw) // SUB):
                    t0, t1 = int(offs[sci]), int(offs[sci + 1])
                    nt = t1 - t0
                    S3 = spool.tile([P, maxt, SUB], bf16, tag="S")
                    nc.vector.tensor_tensor(
                        S3[:, :nt, :],
                        iota[:].unsqueeze(1).broadcast_to([P, nt, SUB]),
                        dst[:, t0:t1].unsqueeze(2).broadcast_to([P, nt, SUB]),
                        is_eq)
                    s3map[sci] = S3

            hmap = {}
            xmap = {}

            def emit_agg(gi):
                # aggregate node group gi into aggT via one-hot matmuls,
                # and prefetch its xb tile (consumed two iterations later)
                n0 = gi * NCHUNK
                nw = min(NCHUNK, RPC - n0)
                xt = xpool.tile([P, NCHUNK // P, EMBED], bf16, tag="x")
                nc.sync.dma_start(
                    out=xt[:, :nw // P, :],
                    in_=xb_d[:, n0 // P:n0 // P + nw // P, :])
                xmap[gi] = xt
                ps4 = psa.tile([FEAT, NCHUNK // P, P], f32, tag="agg")
                for sci in range(n0 // SUB, (n0 + nw) // SUB):
                    si = (sci * SUB - n0) // P
                    so = (sci * SUB) % P
                    t0, t1 = int(offs[sci]), int(offs[sci + 1])
                    S3 = s3map.pop(sci)
                    for t in range(t0, t1):
                        fa = tok_ap(t)
                        nc.tensor.matmul(ps4[:, si, so:so + SUB], fa,
                                         S3[:, t - t0, :],
                                         start=(t == t0), stop=(t == t1 - 1))
                nc.scalar.copy(out=aggT[:, n0:n0 + nw],
                               in_=ps4[:, :nw // P, :])

            def emit_ht(gi):
                # first MLP GEMM + relu for group gi (aggT written one
                # iteration earlier, so the copy is long done)
                n0 = gi * NCHUNK
                nw = min(NCHUNK, RPC - n0)
                hT = hpool.tile([P, 4, NCHUNK], bf16, tag="hT")
                for f in range(4):
                    ph = psh.tile([P, NCHUNK], f32, tag="h")
                    nc.tensor.matmul(ph[:, :nw], wc[:, f * P:(f + 1) * P],
                                     aggT[:, n0:n0 + nw], start=True, stop=True)
                    nc.scalar.activation(out=hT[:, f, :nw], in_=ph[:, :nw],
                                         func=act_relu, bias=b1t[:, f:f + 1])
                hmap[gi] = hT

            def emit_out(gi):
                # out-GEMM + residual injection for group gi (its relus ran
                # a full iteration earlier)
                n0 = gi * NCHUNK
                nw = min(NCHUNK, RPC - n0)
                hT = hmap.pop(gi)
                xt = xmap.pop(gi)
                ot = opool.tile([P, NCHUNK // P, EMBED], bf16, tag="o")
                for t in range(nw // P):
                    po = pso.tile([P, EMBED], f32, tag="po")
                    nc.tensor.matmul(po[:], ident[:], xt[:, t, :],
                                     start=True, stop=False)
                    for k in range(4):
                        nc.tensor.matmul(po[:], hT[:, k, t * P:(t + 1) * P],
                                         w2[:, k, :],
                                         start=False, stop=(k == 3))
                    nc.scalar.copy(out=ot[:, t, :], in_=po[:])
                nc.gpsimd.dma_start(
                    out=out_d[:, n0 // P:n0 // P + nw // P, :],
                    in_=ot[:, :nw // P, :])

            emit_s3_group(0)
            emit_s3_group(1)
            for gi in range(n_groups + 2):
                if gi + 2 < n_groups:
                    emit_s3_group(gi + 2)
                if gi < n_groups:
                    emit_agg(gi)
                if 1 <= gi <= n_groups:
                    emit_ht(gi - 1)
                if gi >= 2:
                    emit_out(gi - 2)
    nc.finalize()
    return nc


def kernel(x, edge_index, rbf_feature, angle_feature, W_edge, b_edge, W1, b1, W2, b2):
    from concourse import bacc, tile, mybir
    from concourse.bass_utils import run_bass_kernel_spmd

    x = np.asarray(x, dtype=np.float32)
    rows = np.asarray(edge_index[0], dtype=np.int64)
    tok_list, dst_list, ntiles, tiles_per_chunk = _host_pack(
        rows, np.asarray(rbf_feature, np.float32),
        np.asarray(angle_feature, np.float32))

    # fold W_aug @ W1 on host (fp64), plus x+b2
    W_aug = np.zeros((FEAT, EMBED), dtype=np.float64)
    W_aug[:RBF + ANG] = np.asarray(W_edge, np.float64)
    W_aug[96] = np.asarray(b_edge, np.float64)
    wc = (W_aug @ np.asarray(W1, np.float64)).astype(np.float32).astype(bfloat16)
    w2d = np.ascontiguousarray(
        np.asarray(W2, np.float32).reshape(4, P, EMBED).transpose(1, 0, 2)
    ).astype(bfloat16)
    b1t = np.ascontiguousarray(np.asarray(b1, np.float32).reshape(4, P).T)
    xb_full = np.zeros((NODES_PAD, EMBED), dtype=bfloat16)
    xb_full[:N_NODES] = (x + np.asarray(b2, np.float32)).astype(bfloat16)

    in_maps = []
    for c in range(N_CORES):
        xbs = xb_full[c * RPC:(c + 1) * RPC]
        in_maps.append({
            "tokens": tok_list[c], "dests": dst_list[c],
            "xb": np.ascontiguousarray(
                xbs.reshape(NCH, P, EMBED).transpose(1, 0, 2)),
            "wc": wc, "w2d": w2d, "b1t": b1t,
        })

    nc = _build_program(ntiles, tiles_per_chunk, mybir, bacc, tile)
    if os.environ.get('DEBUG_SIM') == '1':
        from concourse import bass_interp
        results = []
        for c in range(N_CORES):
            sim = bass_interp.CoreSim(nc)
            for k, v in in_maps[c].items():
                sim.tensor(k)[:] = v
            sim.tensor('out_s')[:] = 0
            sim.simulate()
            results.append({'out_s': np.array(sim.tensor('out_s'))})

        class R:
            pass
        res = R()
        res.results = results
        res.exec_time_ns = None
        res.instructions_and_trace = None
        res.profile_json = None
    else:
        res = run_bass_kernel_spmd(nc, in_maps, list(range(N_CORES)))
    global LAST_RESULT
    LAST_RESULT = res
    out = np.concatenate(
        [res.results[c]["out_s"].astype(np.float32)
         .transpose(1, 0, 2).reshape(RPC, EMBED)
         for c in range(N_CORES)], axis=0)
    return out[:N_NODES]


LAST_RESULT = None
